# revision 1
# baseline (speedup 1.0000x reference)
"""CSPN accelerate (3x3 per-pixel dynamic filter) on 8 Trainium2 NeuronCores.

out[b,0,h,w] = sum_{di,dj in 0..2} K[b,3*di+dj,h,w] * Xpad[b, h+di-1, w+dj-1]
with the center tap (di=dj=1) taking input0 instead of input.

Sharding: pure data-parallel over batch (16 samples -> 2 per core).

The op is DMA-bandwidth bound (the 9-tap kernel tensor dominates traffic), so
inputs are converted to fp16 on the host and all on-chip traffic is fp16;
only PSUM accumulation is fp32. Worst-case rounding is ~2e-3 of the output
absmax, well inside the harness 2e-2 gate, for half the HBM bytes.

Per-core structure (per sample, per 126-row output tile):
  - x / x0 tiles [128, 642|640]: rows [r-1, r+127) in partitions -> the three
    dj (column) shifts are free-dim slices; x0 is row-aligned with x.
  - 3 grouped kernel DMAs, one per di: taps 3di+{0,1,2} are contiguous in
    DRAM, loaded via a transposed access pattern into [128, 3, 640] tiles with
    rows shifted by -di so the per-tap product k*x is input-row aligned.
  - DVE does only the 9 elementwise fp16 multiplies (2x DVE mode); all tap
    summation plus the di row realignment happens on the Tensor engine as 9
    accumulating fp16 shift-matmuls (1 cycle/row) per 320-column chunk into
    single-bank PSUM tiles; ACT downcasts PSUM to fp16 for the store.
Pipeline shaping for the DMA-engine roofline:
  - Output stores of all but the last tile are issued after the whole input
    stream in SP program order, so they land behind every input transfer in
    the DMA-engine queue and fill it while the last tile's compute drains.
  - The last tile's final tap group is loaded as separate per-tap DMAs so the
    last input DMA gates a single multiply, not a whole group.
"""

import numpy as np

import concourse.bacc as bacc
import concourse.bass as bass
import concourse.mybir as mybir
import concourse.tile as tile
from concourse.bass_utils import run_bass_kernel_spmd

F16 = mybir.dt.float16
F32 = mybir.dt.float32

BS, KK, H, W = 16, 9, 480, 640
N_CORES = 8
BPC = BS // N_CORES          # samples per core
P = 128                      # SBUF partitions
TH = P - 2                   # output rows per full tile
W_CHUNKS = ((0, 320), (320, 320))  # single-bank fp32 PSUM chunks
IN_BUFS = 4                  # ring depth for x/x0/k tiles


def build_module() -> bass.Bass:
    nc = bacc.Bacc()
    k_ext = nc.declare_dram_parameter("kern", [BPC, KK, H, W], F16, isOutput=False)
    x_ext = nc.declare_dram_parameter("x", [BPC, 1, H, W], F16, isOutput=False)
    x0_ext = nc.declare_dram_parameter("x0", [BPC, 1, H, W], F16, isOutput=False)
    out_ext = nc.declare_dram_parameter("out", [BPC, 1, H, W], F16, isOutput=True)

    # Shift matrices: M[di][g, h] = 1 iff g == h + di, so that
    # out[h] = sum_g M[g,h] * prod[g] picks input-row-aligned products back
    # into output rows.
    m_np = np.zeros((P, 3, P), np.float16)
    for di in range(3):
        for h in range(TH):
            m_np[h + di, di, h] = 1.0
    m_dram = nc.inline_tensor(m_np, name="shiftm")

    tiles = []
    for r in (126, 252):
        for b in range(BPC):
            tiles.append((b, r, TH))
    for r, th in ((0, TH), (378, H - 378)):
        for b in range(BPC):
            tiles.append((b, r, th))

    with tile.TileContext(nc) as tc:
        with (
            tc.tile_pool(name="consts", bufs=1) as cpool,
            tc.tile_pool(name="kpool", bufs=IN_BUFS) as kpool,
            tc.tile_pool(name="xpool", bufs=IN_BUFS) as xpool,
            tc.tile_pool(name="prodpool", bufs=2) as ppool,
            tc.tile_pool(name="psum", bufs=4, space="PSUM") as psumpool,
        ):
            mtile = cpool.tile([P, 3, P], F16)
            nc.scalar.dma_start(out=mtile[:], in_=m_dram[:])

            stores = []  # deferred (osb, b, r, th)
            for it, (b, r, th) in enumerate(tiles):
                last = it == len(tiles) - 1
                osb = _emit_tile(nc, kpool, xpool, ppool, psumpool, mtile,
                                 k_ext, x_ext, x0_ext, out_ext, b, r, th, last)
                if not last:
                    stores.append((osb, b, r, th))
            # Deferred stores: SP/Pool reach these after issuing every input
            # DMA, so they land behind all input transfers in the DMA-engine
            # queue and fill it while the last tile's compute drains.
            # Alternating issuers keeps the issue rate above the transfer
            # rate (a single sequencer can't sustain one DMA per 450ns).
            for si, (osb, b, r, th) in enumerate(stores):
                eng = nc.sync if si % 2 == 0 else nc.gpsimd
                eng.dma_start(out=out_ext[b, 0, r:r + th, :], in_=osb[:th, :])
    nc.finalize()
    return nc


def _emit_tile(nc, kpool, xpool, ppool, psumpool, mtile,
               k_ext, x_ext, x0_ext, out_ext, b, r, th, last):
    lo = r - 1
    clo, chi = max(lo, 0), min(lo + P, H)
    top_clip, bot_clip = clo > lo, chi < lo + P

    # --- input tile: rows [r-1, r-1+P) of x, zero-padded columns at 0, 641
    # (never DMA-written, so re-zeroed each use) and zero-padded edge rows.
    xt = xpool.tile([P, W + 2], F16)
    nc.gpsimd.memset(xt[:, 0:1], 0.0)
    nc.gpsimd.memset(xt[:, W + 1:W + 2], 0.0)
    if top_clip:
        nc.gpsimd.memset(xt[0:32, :], 0.0)
    if bot_clip:
        nc.gpsimd.memset(xt[96:P, :], 0.0)
    nc.sync.dma_start(out=xt[clo - lo:chi - lo, 1:W + 1], in_=x_ext[b, 0, clo:chi, :])

    # --- center-tap replacement input0, loaded row-aligned with x. Edge pads
    # only keep the read-set initialized; M_1 never selects them.
    x0t = xpool.tile([P, W], F16, tag="x0t")
    if top_clip:
        nc.gpsimd.memset(x0t[0:32, :], 0.0)
    if bot_clip:
        nc.gpsimd.memset(x0t[96:P, :], 0.0)
    nc.sync.dma_start(out=x0t[clo - lo:chi - lo, :], in_=x0_ext[b, 0, clo:chi, :])

    # --- kernel taps: one grouped DMA per di (taps 3di+dj are contiguous in
    # DRAM), rows shifted by -di so the products align to input rows. Edge
    # pads as above: contents unselected by M_di, zeroed to stay initialized.
    kts = []
    for di in range(3):
        klo = r - di
        kclo, kchi = max(klo, 0), min(klo + P, H)
        ktg = kpool.tile([P, 3, W], F16, tag=f"ktg{di}", name=f"ktg{di}")
        if kclo > klo:
            nc.gpsimd.memset(ktg[0:32, :, :], 0.0)
        if kchi < klo + P:
            nc.gpsimd.memset(ktg[96:P, :, :], 0.0)
        if last and di == 2:
            # Per-tap loads: the last-arriving DMA gates a single multiply
            # instead of a whole group.
            for dj in range(3):
                nc.sync.dma_start(
                    out=ktg[kclo - klo:kchi - klo, dj, :],
                    in_=k_ext[b, 3 * di + dj, kclo:kchi, :],
                )
        else:
            nc.sync.dma_start(
                out=ktg[kclo - klo:kchi - klo, :, :],
                in_=k_ext[b, 3 * di:3 * di + 3, kclo:kchi, :].transpose([1, 0, 2]),
            )
        kts.append(ktg)

    # --- 9 fp16 products on DVE; tap-sum + row realignment on PE as fp16
    # shift-matmuls accumulating into fp32 PSUM; ACT downcasts to osb.
    osb = ppool.tile([P, W], F16, tag="osb", bufs=4 * BPC, name="osb")
    psum_c = [psumpool.tile([P, cn], F32, tag=f"ps{ci}", name=f"ps{ci}")
              for ci, (c0, cn) in enumerate(W_CHUNKS)]
    tap = 0
    for di in range(3):
        for dj in range(3):
            prod = ppool.tile([P, W], F16, tag=f"prod{di}{dj}",
                              name=f"prod{di}{dj}")
            in1 = x0t[:, :] if (di == 1 and dj == 1) else xt[:, dj:dj + W]
            nc.vector.tensor_tensor(
                out=prod[:], in0=kts[di][:, dj, :], in1=in1,
                op=mybir.AluOpType.mult)
            for ci, (c0, cn) in enumerate(W_CHUNKS):
                nc.tensor.matmul(
                    out=psum_c[ci][:th, :],
                    lhsT=mtile[:, di, 0:th],
                    rhs=prod[:, c0:c0 + cn],
                    start=(tap == 0),
                    stop=(tap == KK - 1),
                )
            tap += 1
    if last:
        # Parallel PSUM drain on the final tile: ACT and DVE each downcast one
        # chunk, and the two half-stores issue from ACT and SP in parallel so
        # the chain after the last matmul is one copy + one issue deep.
        nc.scalar.copy(out=osb[:th, 0:320], in_=psum_c[0][:th, :])
        nc.vector.tensor_copy(out=osb[:th, 320:640], in_=psum_c[1][:th, :])
        nc.scalar.dma_start(out=out_ext[b, 0, r:r + th, :], in_=osb[:th, :])
    else:
        for ci, (c0, cn) in enumerate(W_CHUNKS):
            nc.scalar.copy(out=osb[:th, c0:c0 + cn], in_=psum_c[ci][:th, :])
    return osb


_NC_CACHE = None


def _get_module():
    global _NC_CACHE
    if _NC_CACHE is None:
        _NC_CACHE = build_module()
    return _NC_CACHE


def kernel(**inputs: np.ndarray) -> np.ndarray:
    kern = np.asarray(inputs["kernel"], dtype=np.float16)
    x = np.asarray(inputs["input"], dtype=np.float16)
    x0 = np.asarray(inputs["input0"], dtype=np.float16)
    assert kern.shape == (BS, KK, H, W), kern.shape

    nc = _get_module()
    in_maps = [
        {
            "kern": np.ascontiguousarray(kern[c * BPC:(c + 1) * BPC]),
            "x": np.ascontiguousarray(x[c * BPC:(c + 1) * BPC]),
            "x0": np.ascontiguousarray(x0[c * BPC:(c + 1) * BPC]),
        }
        for c in range(N_CORES)
    ]
    res = run_bass_kernel_spmd(nc, in_maps, list(range(N_CORES)))
    out = np.concatenate([res.results[c]["out"] for c in range(N_CORES)], axis=0)
    return out.astype(np.float32)

